# revision 22
# baseline (speedup 1.0000x reference)
"""Trainium2 Bass kernel for nn_DotProductAttention (B=8, LQ=LK=4096, F=64).

Reference computation:
    q = query @ wq.T + bq ; k = key @ wk.T + bk ; v = value @ wv.T + bv
    scores = einsum('bkf,bqf->bkq', k, q)
    attn = softmax(scores, axis=-1)           # over q positions
    out = einsum('bkq,bqf->bkf', attn, v)

Strategy: batch b -> core b (8 cores, no cross-core communication).

Algebraic folding (host side, O(L*F) prep only -- all O(L^2) work on device):
    scores[k,q] = (wk x_k + bk).(wq x_q + bq)
                = x_q^T (wq^T wk) x_k + x_q^T (wq^T bk) + [per-k term]
    The per-k term is constant along the softmax axis (q) and cancels in the
    softmax, so with M = wq^T wk, c = wq^T bk the transposed scores are
        S^T[q,k] = query[q,:] @ ktil[:,k],   ktil = M @ key^T + c   (host)
    Softmax rows sum to 1, so the v-projection commutes with attention:
        out = (attn @ value) @ wv.T + bv
    exp() needs no max-subtraction: |S| < ~70 so exp fits fp32/bf16 range.
    U^T = [value | 1]^T @ exp(S^T) accumulates in PSUM; its last row is the
    softmax denominator l. The tiny output projection (U/l) @ wv.T + bv runs
    on host in fp32.

Device loop (per core): for each 512-wide k-chunk, sweep the 32 q-blocks:
one N=512 fp16 scores matmul per j-block (alternating 64-row PE groups so
adjacent matmuls row-tile concurrently) filling [128,1536] PSUM supertiles
of 3x512 slots; one ACT exp per supertile into bf16 SBUF; P@V lags LAG
j-steps behind, split into two K=64 row-group matmuls with separate
accumulator banks (concurrent, cross-group LDW overlap). The ACT engine
(16.7M exp @ 128 lanes x 1.2GHz) is saturated and bounds the runtime.
"""

import numpy as np
import ml_dtypes

import concourse.mybir as mybir
import concourse.tile as tile
from concourse import bacc
from concourse.bass_utils import run_bass_kernel_spmd
from concourse.vector_clock import ScopedClock


class _FastExitTileContext(tile.TileContext):
    """TileContext whose exit skips the second all-engine barrier.

    The final barrier only orders the gpsimd semaphore-clears against the
    other engines' completion; NEFF execution completion already waits for
    every engine's last instruction, and the clears still run, so repeated
    executions stay correct. Saves ~2-3us of kernel tail.
    """

    def _drain_and_barrier(self, tick_clock, wait_clock):
        drain_inst = self.nc.sync.drain()
        wait_clock.add_sem_waits(
            drain_inst.ins, ScopedClock({None: tick_clock.global_clock})
        )
        self.nc.all_engine_barrier()
        popped = self.nc._tile_sem_poison_stack.pop()
        assert popped is self._sem_poison
        self.nc.clear_and_free_semaphores(list(self.sems.allocated().values()))

F32 = mybir.dt.float32
F16 = mybir.dt.float16
BF16 = mybir.dt.bfloat16

L = 4096          # sequence length (both q and k)
F = 64            # feature dim
NBLK = L // 128   # 32 position blocks
CHW = 512         # k-chunk width


def build_nc():
    nc = bacc.Bacc(None, target_bir_lowering=False)

    xqT = nc.dram_tensor("xqT", [128, L // 2], F16, kind="ExternalInput")
    ktil = nc.dram_tensor("ktil", [128, L], F16, kind="ExternalInput")
    vaug = nc.dram_tensor("vaug", [128, NBLK * (F + 1)], BF16, kind="ExternalInput")
    uout = nc.dram_tensor("uout", [F + 1, L], F32, kind="ExternalOutput")

    Exp = mybir.ActivationFunctionType.Exp

    with _FastExitTileContext(nc) as tc:
        with (
            tc.tile_pool(name="persist", bufs=1) as persist,
            tc.tile_pool(name="pt", bufs=6) as ptpool,
            tc.tile_pool(name="utbf", bufs=2) as utbfpool,
            tc.tile_pool(name="ps_st", bufs=2, space="PSUM") as ps_st,
            tc.tile_pool(name="ps_ut", bufs=2, space="PSUM") as ps_ut,
        ):
            # Split DMAs so the first iterations' inputs land early; the
            # j=0 row-half quarters go absolutely first.
            xqT_sb = persist.tile([128, L // 2], F16)
            ktil_sb = persist.tile([128, L], F16)
            vaug_sb = persist.tile([128, NBLK * (F + 1)], BF16)
            nc.sync.dma_start(xqT_sb[0:64, 0:128], xqT[0:64, 0:128])
            nc.sync.dma_start(ktil_sb[0:64, 0:CHW], ktil[0:64, 0:CHW])
            nc.sync.dma_start(xqT_sb[64:128, 0:128], xqT[64:128, 0:128])
            nc.sync.dma_start(ktil_sb[64:128, 0:CHW], ktil[64:128, 0:CHW])
            nc.sync.dma_start(vaug_sb[:, 0:2 * (F + 1)], vaug[:, 0:2 * (F + 1)])
            nc.sync.dma_start(xqT_sb[:, 128:], xqT[:, 128:])
            nc.sync.dma_start(vaug_sb[:, 2 * (F + 1):], vaug[:, 2 * (F + 1):])
            nc.sync.dma_start(ktil_sb[:, CHW:], ktil[:, CHW:])

            # ---- main loop ----
            # Scores for consecutive j-blocks (alternating 64-row groups, so
            # adjacent matmuls overlap via row tiling) fill [128,1536] PSUM
            # supertiles of 3 x 512 slots; one ACT exp per supertile. P@V
            # lags scores by LAG j-steps so the PE never waits on ACT.
            GRP = 3
            LAG = 8
            NCH = 8
            uts = {}
            sts = {}
            pts = {}

            # per-chunk slot-group sizes (sum = NBLK); chunk 0 front-loads a
            # 1-slot group so the first exp fires as early as possible.
            group_sizes = {0: [1, 1] + [GRP] * 10}
            for _c in range(1, NCH):
                group_sizes[_c] = [GRP] * 10 + [2]
            jmap = {}
            for _c in range(NCH):
                _j = 0
                for _g, _s in enumerate(group_sizes[_c]):
                    for _off in range(_s):
                        jmap[(_c, _j)] = (_g, _off, _s)
                        _j += 1

            def emit_scores(c, j):
                g, off, slots = jmap[(c, j)]
                if off == 0:
                    sts[(c, g)] = ps_st.tile([128, 512 * slots], F32,
                                             name="st", tag="st")
                st = sts[(c, g)]
                rh = 64 * (j % 2)
                qcols = slice(128 * (j // 2), 128 * (j // 2 + 1))
                kcols = slice(CHW * c, CHW * (c + 1))
                nc.tensor.matmul(st[:, 512 * off: 512 * (off + 1)],
                                 xqT_sb[rh:rh + 64, qcols],
                                 ktil_sb[rh:rh + 64, kcols],
                                 start=True, stop=True, tile_position=(rh, 0))
                if off == slots - 1:
                    pt = ptpool.tile([128, 512 * slots], BF16,
                                     name="pt", tag="pt")
                    nc.scalar.activation(pt[:], sts.pop((c, g))[:], Exp)
                    pts[(c, g)] = pt

            def emit_pav(c, j):
                if j == 0:
                    uts[c] = (ps_ut.tile([F + 1, CHW], F32, name="utl", tag="ut"),
                              ps_ut.tile([F + 1, CHW], F32, name="uth", tag="ut"))
                utl, uth = uts[c]
                g, off, slots = jmap[(c, j)]
                pt = pts[(c, g)]
                ksl = slice(512 * off, 512 * (off + 1))
                vsl = slice((F + 1) * j, (F + 1) * (j + 1))
                # contraction split into two row-groups: concurrent on the PE
                # array (separate accumulator banks), LDWs overlap cross-group.
                nc.tensor.matmul(utl[:], vaug_sb[0:64, vsl], pt[0:64, ksl],
                                 start=(j == 0), stop=(j == NBLK - 1),
                                 tile_position=(0, 0))
                nc.tensor.matmul(uth[:], vaug_sb[64:128, vsl], pt[64:128, ksl],
                                 start=(j == 0), stop=(j == NBLK - 1),
                                 tile_position=(64, 0))
                if off == slots - 1:
                    pts.pop((c, g))

            def emit_epilogue(c):
                utl, uth = uts.pop(c)
                utbf = utbfpool.tile([F + 1, CHW], F32)
                nc.vector.tensor_copy(utbf[:], utl[:])
                nc.vector.tensor_tensor(utbf[:], uth[:], utbf[:],
                                        mybir.AluOpType.add)
                nc.sync.dma_start(uout[:, CHW * c: CHW * (c + 1)], utbf[:])

            NTOT = NCH * NBLK
            for gstep in range(NTOT + LAG):
                if gstep < NTOT:
                    emit_scores(gstep // NBLK, gstep % NBLK)
                if gstep >= LAG:
                    pc, pj = (gstep - LAG) // NBLK, (gstep - LAG) % NBLK
                    emit_pav(pc, pj)
                    if pj == NBLK - 1:
                        emit_epilogue(pc)

    nc.compile()
    return nc


def host_pack(query_b, key_b, value_b, M, c):
    """Per-batch device-input packing (numpy, O(L*F))."""
    qT = query_b.T.reshape(F, L // 256, 2, 128)
    xqT = np.ascontiguousarray(                                       # [128, L/2]
        np.concatenate([qT[:, :, 0, :], qT[:, :, 1, :]], axis=0)
        .reshape(128, L // 2)).astype(np.float16)
    kt = (M @ key_b.T + c[:, None]).astype(np.float16)                # [64, L]
    ktil = np.ascontiguousarray(np.concatenate([kt, kt], axis=0))     # [128, L]
    v3 = value_b.reshape(NBLK, 128, F).transpose(1, 0, 2)             # [128, NBLK, F]
    vaug = np.ones((128, NBLK, F + 1), np.float32)
    vaug[:, :, 0:F] = v3
    vaug_bf = vaug.reshape(128, NBLK * (F + 1)).astype(ml_dtypes.bfloat16)
    return xqT, ktil, np.ascontiguousarray(vaug_bf)


def host_consts(wq, bq, wk, bk, wv, bv):
    wq64 = wq.astype(np.float64)
    M = (wq64.T @ wk.astype(np.float64)).astype(np.float32)
    c = (wq64.T @ bk.astype(np.float64)).astype(np.float32)
    return M, c


_NC = None


def kernel(**inputs):
    out, _ = run_kernel(inputs)
    return out


def run_kernel(inputs, **spmd_kwargs):
    global _NC
    if _NC is None:
        _NC = build_nc()

    query = np.asarray(inputs["query"], np.float32)
    key = np.asarray(inputs["key"], np.float32)
    value = np.asarray(inputs["value"], np.float32)
    wv = np.asarray(inputs["wv"], np.float32)
    bv = np.asarray(inputs["bv"], np.float32)
    M, c = host_consts(
        np.asarray(inputs["wq"], np.float32), np.asarray(inputs["bq"], np.float32),
        np.asarray(inputs["wk"], np.float32), np.asarray(inputs["bk"], np.float32),
        wv, bv)

    B = query.shape[0]
    in_maps = []
    for b in range(B):
        xqT, ktil, vaug = host_pack(query[b], key[b], value[b], M, c)
        in_maps.append({"xqT": xqT, "ktil": ktil, "vaug": vaug})
    res = run_bass_kernel_spmd(_NC, in_maps, core_ids=list(range(B)), **spmd_kwargs)
    outs = []
    for b in range(B):
        u = res.results[b]["uout"]              # [65, L] fp32: U^T rows + l row
        ut = (u[0:F, :] / u[F:F + 1, :]).T      # [L, F] normalized attention @ value
        outs.append(ut @ wv.T + bv)             # host fp32 epilogue projection
    out = np.stack(outs).astype(np.float32)
    return out, res
